# revision 35
# baseline (speedup 1.0000x reference)
"""Trainium2 Bass kernel for nn_DirectionalAttention (B=8,S=1024,D=1024,H=16).

Sharding: pure data-parallel over batch — 8 batch elements onto 8 NeuronCores,
zero collectives; each core runs the full attention layer for one batch
element. Host pre-transposes activations/weights (no on-device input
transposes) and folds the out-proj bias as bo' = bv@wo.T + bo into the
residual input (valid because softmax rows sum to 1).

All matmul operands are fp16 (PSUM accumulates fp32; measured end-to-end
error ~5e-4 relmax on attn, ~3e-4 on y). Per core:
  QT = wq @ x^T [D,S], KT likewise, V = v @ wv^T [S,D]  (k-outer, 8 live
    PSUM banks, weights resident, activations streamed twice)
  Attention runs over head PAIRS (2j, 2j+1) whose DK=64 slices sit at PE
  row groups 0:64/64:128, interleaving their K=64 matmuls for array-level
  concurrency, with a depth-1 pair skew so PE stays busy across ACT exp:
    pass1: scores[q,k] -> additive mask (mixed blocks) -> ACT
      Exp(scale=1/8, accum_out=rowsum) -> reciprocal -> normalize (DVE 2x)
      -> DMA out. Fully-masked blocks are skipped entirely (output buffers
      arrive pre-zeroed through the PJRT donation path).
    scoresT is computed directly on PE by swapping matmul operands (no
      transposes), exp'd into fp16 attnT (unnormalized).
    AV is computed transposed, AVT[d,q] = Vh.T @ attnT — exactly the lhsT
      layout the output projection needs; the softmax 1/rowsum is applied
      at AV-copyback via a PE-transposed rinv row broadcast with gpsimd.
      Odd heads reach AVT partitions 64:128 via a small SBUF->SBUF DMA
      bounce (matmul PSUM dst must start at partition 0).
  y = LN(AVT.T @ woT + query + bo') * gamma + beta  (bn_stats/bn_aggr).

The traced program adapts to the causal_mask block structure at build time
(all-ones / all-zero / mixed 128x128 blocks), so any block-structured mask
compiles to a specialized kernel; the tril mask gives the causal program.

Self-contained: hardcodes shapes; reads nothing from /root/problem.
"""

import numpy as np

import concourse.bacc as bacc
import concourse.bass as bass
import concourse.mybir as mybir
from concourse.tile import TileContext
from concourse.masks import make_identity
from concourse.bass_utils import run_bass_kernel_spmd

B, S, D, H = 8, 1024, 1024, 16
DK = D // H
P = 128
NT = S // P
LN_EPS = 1e-5
NEG = -1e9

MM_DT = mybir.dt.float16
F32 = mybir.dt.float32
FP16 = mybir.dt.float16
AFT = mybir.ActivationFunctionType

_CACHE: dict = {}


def _classify_blocks(mask: np.ndarray):
    kinds, mixed = {}, {}
    for qi in range(NT):
        for ki in range(NT):
            blk = mask[qi * P:(qi + 1) * P, ki * P:(ki + 1) * P]
            if (blk != 0).all():
                kinds[qi, ki] = "one"
            elif (blk == 0).all():
                kinds[qi, ki] = "zero"
            else:
                kinds[qi, ki] = "mix"
                mixed[qi, ki] = np.where(blk == 0, np.float32(NEG),
                                         np.float32(0.0))
    return kinds, mixed


def _build(kinds, n_mixed):
    nc = bacc.Bacc("TRN2", target_bir_lowering=False, debug=False,
                   num_devices=B)
    dp = nc.declare_dram_parameter
    xT = dp("xT", [D, S], F32, isOutput=False)
    kTd = dp("kT", [D, S], F32, isOutput=False)
    vT = dp("vT", [D, S], F32, isOutput=False)
    xq = dp("xq", [S, D], F32, isOutput=False)
    wqT = dp("wqT", [D, D], F32, isOutput=False)
    wkT = dp("wkT", [D, D], F32, isOutput=False)
    wvT = dp("wvT", [D, D], F32, isOutput=False)
    woT = dp("woT", [D, D], F32, isOutput=False)
    bq8 = dp("bq8", [P, NT], F32, isOutput=False)
    bk8 = dp("bk8", [P, NT], F32, isOutput=False)
    gam = dp("gam", [1, D], F32, isOutput=False)
    bet = dp("bet", [1, D], F32, isOutput=False)
    mixm = dp("mixm", [max(n_mixed, 1), P, P], F32, isOutput=False)
    mixmT = dp("mixmT", [max(n_mixed, 1), P, P], F32, isOutput=False)
    y_d = dp("y", [S, D], F32, isOutput=True)
    attn_d = dp("attn", [H, S, S], F32, isOutput=True)

    kblocks = {qi: [ki for ki in range(NT) if kinds[qi, ki] != "zero"]
               for qi in range(NT)}
    for qi in range(NT):
        assert kblocks[qi], "fully-masked row tile: softmax undefined"
    mix_idx = {}
    for qi in range(NT):
        for ki in range(NT):
            if kinds[qi, ki] == "mix":
                mix_idx[qi, ki] = len(mix_idx)

    with TileContext(nc) as tc:
      with tc.tile_pool(name="consts", bufs=1) as cp, \
           tc.tile_pool(name="stat", bufs=6) as st:
        ident = cp.tile([P, P], F32)
        make_identity(nc, ident)
        gam_b = cp.tile([P, D], F32)
        bet_b = cp.tile([P, D], F32)
        with tc.tile_pool(name="rowstage", bufs=1) as rsp:
            grow = rsp.tile([1, D], F32, tag="grow")
            brow = rsp.tile([1, D], F32, tag="brow")
            nc.sync.dma_start(grow, gam[:, :])
            nc.sync.dma_start(brow, bet[:, :])
            nc.gpsimd.partition_broadcast(gam_b, grow)
            nc.gpsimd.partition_broadcast(bet_b, brow)
        bq_sb = cp.tile([P, NT], F32)
        bk_sb = cp.tile([P, NT], F32)
        nc.sync.dma_start(bq_sb, bq8[:, :])
        nc.sync.dma_start(bk_sb, bk8[:, :])
        eps_sb = cp.tile([P, 1], F32)
        nc.vector.memset(eps_sb, LN_EPS)
        masks_sb = cp.tile([P, max(n_mixed, 1), P], F32)
        nc.sync.dma_start(masks_sb, mixm[:, :, :].rearrange("n p m -> p n m"))
        masksT_sb = cp.tile([P, max(n_mixed, 1), P], F32)
        nc.sync.dma_start(masksT_sb, mixmT[:, :, :].rearrange("n p m -> p n m"))

        with tc.tile_pool(name="avt", bufs=1) as ap:
          AVT = ap.tile([P, NT, S], FP16)

          with tc.tile_pool(name="qkv", bufs=1) as qp:
            QT = qp.tile([P, NT, S], MM_DT)
            KT = qp.tile([P, NT, S], MM_DT)
            V = qp.tile([P, NT, D], FP16)

            # ============ projections (k-outer, 8 live psum banks) ========
            with tc.tile_pool(name="wres", bufs=1) as wr, \
                 tc.tile_pool(name="praw", bufs=3) as raw, \
                 tc.tile_pool(name="pcast", bufs=3) as cst, \
                 tc.tile_pool(name="ppsum", bufs=8, space="PSUM") as pps:

                def project(w_dram, x_dram, out_sb, bias_sb, w_is_lhsT):
                    w_r = wr.tile([P, NT, D], MM_DT, tag="wres")
                    for k in range(NT):
                        wt = raw.tile([P, D], F32, tag="rawW")
                        nc.sync.dma_start(wt, w_dram[bass.ds(k * P, P), :])
                        nc.vector.tensor_copy(w_r[:, k], wt)
                    # K=64 row-group bank pairs (A rows 0:64, B 64:128),
                    # interleaved across banks for array concurrency; n-split
                    # outer loop keeps activation streaming at 2 passes.
                    for mg in range(2):
                      for n in range(2):
                        pss = [pps.tile([P, 512], F32, tag="pj",
                                        name=f"pj{i}")
                               for i in range(8)]
                        for k in range(NT):
                            half = n if w_is_lhsT else mg
                            xt = raw.tile([P, 512], F32, tag="rawX")
                            nc.sync.dma_start(
                                xt, x_dram[bass.ds(k * P, P),
                                           half * 512:(half + 1) * 512])
                            xc = cst.tile([P, 512], MM_DT, tag="castX")
                            nc.vector.tensor_copy(xc, xt)
                            for mi in range(4):
                                m = mg * 4 + mi
                                for g in range(2):
                                    gs = slice(64 * g, 64 * g + 64)
                                    if w_is_lhsT:
                                        lhsT = w_r[gs, k, m * P:(m + 1) * P]
                                        rhs = xc[gs, :]
                                    else:
                                        lhsT = xc[gs, mi * P:(mi + 1) * P]
                                        rhs = w_r[gs, k,
                                                  n * 512:(n + 1) * 512]
                                    nc.tensor.matmul(
                                        pss[mi * 2 + g], lhsT, rhs,
                                        start=(k == 0),
                                        stop=(k == NT - 1))
                        for mi in range(4):
                            m = mg * 4 + mi
                            dst = out_sb[:, m, n * 512:(n + 1) * 512]
                            pa_ = pss[mi * 2]
                            pb_ = pss[mi * 2 + 1]
                            if bias_sb is not None:
                                nc.vector.tensor_scalar_add(
                                    dst, pa_, bias_sb[:, m:m + 1])
                            else:
                                nc.vector.tensor_copy(dst, pa_)
                            nc.vector.tensor_add(dst, dst, pb_)

                project(wqT, xT, QT, bq_sb, True)
                project(wkT, kTd, KT, bk_sb, True)
                project(wvT, vT, V, None, False)

            # ===================== attention ==============================
            with tc.tile_pool(name="atile", bufs=1) as tp, \
                 tc.tile_pool(name="awork", bufs=3) as wkp, \
                 tc.tile_pool(name="psc", bufs=2, space="PSUM") as psc, \
                 tc.tile_pool(name="pav", bufs=2, space="PSUM") as pavp:
                attnT = tp.tile([P, NT, S], FP16)
                rinv_hq = tp.tile([P, H * NT], F32)
                zero_f = tp.tile([P, S], F32, tag="zf")
                nc.vector.memset(zero_f, 0.0)
                for ki in range(NT):
                    if any(kinds[qi, ki] == "zero" for qi in range(NT)):
                        nc.vector.tensor_copy(attnT[:, ki], zero_f)

                # Head-PAIR loop: heads (2j, 2j+1) sit at PE row
                # groups 0:64 / 64:128, so interleaving their K=64
                # matmuls runs them concurrently in the array (measured
                # ~2x). Depth-1 skew keeps PE dense across ACT exp
                # latency; attnT is (pair x skew) buffered.
                attnT_bufs = [attnT] + [
                    tp.tile([P, NT, S], FP16, name=f"attnT{i}")
                    for i in range(1, 4)]
                for buf in attnT_bufs:
                    for ki in range(NT):
                        if any(kinds[qi, ki] == "zero"
                               for qi in range(NT)):
                            nc.vector.tensor_copy(buf[:, ki], zero_f)

                def emit_pass1_pair(j):
                    for qi in range(NT):
                        kbs = kblocks[qi]
                        wmax = (max(kbs) + 1) * P
                        pss = []
                        for h in (2 * j, 2 * j + 1):
                            ht, hp = divmod(h * DK, P)
                            ps = psc.tile([P, S], F32, tag="psc",
                                          name="ps")
                            pss.append(ps)
                        # interleave the two heads' score matmuls
                        runs = []
                        for g in range(2):
                            grp = [ki for ki in kbs
                                   if g * 4 <= ki < g * 4 + 4]
                            while grp:
                                a = grp[0]
                                b2 = a
                                while b2 + 1 in grp:
                                    b2 += 1
                                grp = [x for x in grp if x > b2]
                                runs.append((a, b2))
                        for (a, b2) in runs:
                            for i, h in enumerate((2 * j, 2 * j + 1)):
                                ht, hp = divmod(h * DK, P)
                                nc.tensor.matmul(
                                    pss[i][:, a * P:(b2 + 1) * P],
                                    QT[hp:hp + DK, ht,
                                       qi * P:(qi + 1) * P],
                                    KT[hp:hp + DK, ht,
                                       a * P:(b2 + 1) * P],
                                    start=True, stop=True)
                        for i, h in enumerate((2 * j, 2 * j + 1)):
                            ps = pss[i]
                            for ki in range(wmax // P):
                                if ki not in kbs:
                                    nc.vector.memset(
                                        ps[:, ki * P:(ki + 1) * P], NEG)
                                elif kinds[qi, ki] == "mix":
                                    nc.vector.tensor_add(
                                        ps[:, ki * P:(ki + 1) * P],
                                        ps[:, ki * P:(ki + 1) * P],
                                        masks_sb[:, mix_idx[qi, ki]])
                            ex = wkp.tile([P, S], F32, tag="exp",
                                          name="ex")
                            rs = st.tile([P, 1], F32, tag="rs", name="rs")
                            at = wkp.tile([P, S], F32, tag="attn",
                                          name="at")
                            nc.scalar.activation(
                                ex[:, :wmax], ps[:, :wmax], AFT.Exp,
                                scale=0.125, accum_out=rs)
                            hq = h * NT + qi
                            nc.vector.reciprocal(rinv_hq[:, hq:hq + 1], rs)
                            nc.vector.tensor_scalar_mul(
                                at[:, :wmax], ex[:, :wmax],
                                rinv_hq[:, hq:hq + 1])
                            nc.sync.dma_start(
                                attn_d[h, bass.ds(qi * P, P), 0:wmax],
                                at[:, :wmax])

                def emit_rbc_pair(j):
                    pstr = pavp.tile([P, 512], F32, tag="psT",
                                     name="pstr")
                    nc.tensor.transpose(pstr[:, 0:P], rinv_hq, ident)
                    rt = tp.tile([P, P], F32, tag="rinvT", name="rt",
                                 bufs=2)
                    nc.vector.tensor_copy(rt, pstr[:, 0:P])
                    rbcs = {}
                    for h in (2 * j, 2 * j + 1):
                        for c in range(2):
                            hq0 = h * NT + 4 * c
                            rstage = wkp.tile([1, 512], F32, tag="rstg",
                                              name="rstage")
                            nc.gpsimd.dma_start(rstage, rt[hq0:hq0 + 4, :])
                            rbc = wkp.tile([DK, 512], F32, tag="rbc",
                                           name="rbc", bufs=8)
                            nc.gpsimd.partition_broadcast(rbc, rstage)
                            rbcs[h, c] = rbc
                    return rbcs

                def emit_scoresT_pair(j, bufs2):
                    for c in range(2):
                        c0 = c * 512
                        for ki in range(NT):
                            qs = [qi for qi in range(4 * c, 4 * c + 4)
                                  if kinds[qi, ki] != "zero"]
                            if not qs:
                                continue
                            if qs != list(range(qs[0], qs[-1] + 1)):
                                raise NotImplementedError(
                                    "non-contiguous valid q-blocks")
                            qlo = qs[0] * P - c0
                            qhi = (qs[-1] + 1) * P - c0
                            psTs = []
                            for i, h in enumerate((2 * j, 2 * j + 1)):
                                ht, hp = divmod(h * DK, P)
                                psT = pavp.tile([P, 512], F32, tag="psT",
                                                name="psT")
                                nc.tensor.matmul(
                                    psT[:, qlo:qhi],
                                    KT[hp:hp + DK, ht,
                                       ki * P:(ki + 1) * P],
                                    QT[hp:hp + DK, ht,
                                       c0 + qlo:c0 + qhi],
                                    start=True, stop=True)
                                psTs.append(psT)
                            for i, h in enumerate((2 * j, 2 * j + 1)):
                                psT = psTs[i]
                                for qi in qs:
                                    a = qi * P - c0
                                    if kinds[qi, ki] == "mix":
                                        nc.vector.tensor_add(
                                            psT[:, a:a + P],
                                            psT[:, a:a + P],
                                            masksT_sb[:, mix_idx[qi, ki]])
                                nc.scalar.activation(
                                    bufs2[i][:, ki, c0 + qlo:c0 + qhi],
                                    psT[:, qlo:qhi], AFT.Exp, scale=0.125)

                def emit_av_pair(j, bufs2, rbcs):
                    for c in range(2):
                        kis = sorted({k for qi in range(4 * c, 4 * c + 4)
                                      for k in kblocks[qi]})
                        pas = []
                        for i, h in enumerate((2 * j, 2 * j + 1)):
                            pa = pavp.tile([DK, 512], F32, tag="pav",
                                           name="pa", bufs=2)
                            pas.append(pa)
                        for idx, ki in enumerate(kis):
                            for i, h in enumerate((2 * j, 2 * j + 1)):
                                nc.tensor.matmul(
                                    pas[i],
                                    V[:, ki, h * DK:(h + 1) * DK],
                                    bufs2[i][:, ki,
                                             c * 512:(c + 1) * 512],
                                    start=(idx == 0),
                                    stop=(idx == len(kis) - 1))
                        for i, h in enumerate((2 * j, 2 * j + 1)):
                            ht, hp = divmod(h * DK, P)
                            if hp == 0:
                                nc.vector.tensor_mul(
                                    AVT[0:DK, ht,
                                        c * 512:(c + 1) * 512],
                                    pas[i], rbcs[h, c])
                            else:
                                bounce = wkp.tile([DK, 512], FP16,
                                                  tag="bnc", name="bounce")
                                nc.vector.tensor_mul(
                                    bounce, pas[i], rbcs[h, c])
                                nc.sync.dma_start(
                                    AVT[hp:hp + DK, ht,
                                        c * 512:(c + 1) * 512], bounce)

                prev = None
                for j in range(H // 2):
                    bufs2 = [attnT_bufs[2 * (j % 2)],
                             attnT_bufs[2 * (j % 2) + 1]]
                    emit_pass1_pair(j)
                    rbcs = emit_rbc_pair(j)
                    emit_scoresT_pair(j, bufs2)
                    if prev is not None:
                        emit_av_pair(*prev)
                    prev = (j, bufs2, rbcs)
                assert prev is not None
                emit_av_pair(*prev)

          # ============== output projection + residual + LN ===============
          with tc.tile_pool(name="ores", bufs=1) as owr, \
               tc.tile_pool(name="oraw", bufs=3) as oraw, \
               tc.tile_pool(name="owork", bufs=2) as ow, \
               tc.tile_pool(name="opsum", bufs=4, space="PSUM") as ops:
            wo_r = owr.tile([P, NT, D], FP16)
            for k in range(NT):
                wt = oraw.tile([P, D], F32, tag="rawW")
                nc.sync.dma_start(wt, woT[bass.ds(k * P, P), :])
                nc.vector.tensor_copy(wo_r[:, k], wt)
            for m in range(NT):
                xr = oraw.tile([P, D], F32, tag="rawX")
                nc.sync.dma_start(xr, xq[bass.ds(m * P, P), :])
                y0 = ow.tile([P, D], F32, tag="y0")
                for n in range(2):
                    psa = ops.tile([P, 512], F32, tag="oj", name="psa")
                    psb = ops.tile([P, 512], F32, tag="oj", name="psb")
                    for k in range(NT):
                        for g, pdst in ((0, psa), (1, psb)):
                            gs = slice(64 * g, 64 * g + 64)
                            nc.tensor.matmul(
                                pdst, AVT[gs, k, m * P:(m + 1) * P],
                                wo_r[gs, k, n * 512:(n + 1) * 512],
                                start=(k == 0), stop=(k == NT - 1))
                    nc.vector.tensor_add(
                        y0[:, n * 512:(n + 1) * 512], psa,
                        xr[:, n * 512:(n + 1) * 512])
                    nc.vector.tensor_add(
                        y0[:, n * 512:(n + 1) * 512],
                        y0[:, n * 512:(n + 1) * 512], psb)
                stats = st.tile([P, 2, 6], F32, tag="bns")
                mv = st.tile([P, 2], F32, tag="mv")
                for sg in range(2):
                    nc.vector.bn_stats(stats[:, sg],
                                       y0[:, sg * 512:(sg + 1) * 512])
                nc.vector.bn_aggr(mv, stats)
                sd = st.tile([P, 1], F32, tag="sd")
                nc.scalar.activation(sd, mv[:, 1:2], AFT.Sqrt, bias=eps_sb)
                rsd = st.tile([P, 1], F32, tag="rsd")
                nc.vector.reciprocal(rsd, sd)
                y1 = ow.tile([P, D], F32, tag="y1")
                nc.vector.tensor_scalar(
                    y1, y0, scalar1=mv[:, 0:1], scalar2=rsd,
                    op0=mybir.AluOpType.subtract, op1=mybir.AluOpType.mult)
                nc.vector.tensor_mul(y1, y1, gam_b)
                nc.gpsimd.tensor_add(y1, y1, bet_b)
                nc.sync.dma_start(y_d[bass.ds(m * P, P), :], y1)

    nc.compile()
    return nc


def _prepare(query, key, value, causal_mask, wq, bq, wk, bk, wv, bv,
             wo, bo, gamma, beta):
    query = np.asarray(query, np.float32)
    key = np.asarray(key, np.float32)
    value = np.asarray(value, np.float32)
    mask = np.asarray(causal_mask)
    wq, bq = np.asarray(wq, np.float32), np.asarray(bq, np.float32)
    wk, bk = np.asarray(wk, np.float32), np.asarray(bk, np.float32)
    wv, bv = np.asarray(wv, np.float32), np.asarray(bv, np.float32)
    wo, bo = np.asarray(wo, np.float32), np.asarray(bo, np.float32)
    gamma, beta = np.asarray(gamma, np.float32), np.asarray(beta, np.float32)

    kinds, mixed = _classify_blocks(mask)
    key_sig = tuple(sorted(kinds.items()))
    if key_sig not in _CACHE:
        _CACHE[key_sig] = _build(kinds, max(len(mixed), 1))
    nc = _CACHE[key_sig]

    mix_list = [mixed[qi, ki] for qi in range(NT) for ki in range(NT)
                if kinds[qi, ki] == "mix"]
    mixm = (np.stack(mix_list) if mix_list
            else np.zeros((1, P, P), np.float32))
    mixmT = np.ascontiguousarray(mixm.transpose(0, 2, 1))
    bo_p = (bv @ wo.T + bo).astype(np.float32)
    shared = {
        "wqT": np.ascontiguousarray(wq.T), "wkT": np.ascontiguousarray(wk.T),
        "wvT": np.ascontiguousarray(wv.T), "woT": np.ascontiguousarray(wo.T),
        "bq8": np.ascontiguousarray(bq.reshape(NT, P).T),
        "bk8": np.ascontiguousarray(bk.reshape(NT, P).T),
        "gam": np.ascontiguousarray(gamma.reshape(1, D)),
        "bet": np.ascontiguousarray(beta.reshape(1, D)),
        "mixm": np.ascontiguousarray(mixm),
        "mixmT": mixmT,
    }
    in_maps = [{
        "xT": np.ascontiguousarray(query[b].T),
        "kT": np.ascontiguousarray(key[b].T),
        "vT": np.ascontiguousarray(value[b].T),
        "xq": np.ascontiguousarray(query[b] + bo_p),
        **shared,
    } for b in range(B)]
    return nc, in_maps


def kernel(**inputs):
    nc, in_maps = _prepare(**inputs)
    res = run_bass_kernel_spmd(nc, in_maps, list(range(B)))
    y = np.stack([res.results[b]["y"] for b in range(B)])
    attn = np.stack([res.results[b]["attn"] for b in range(B)])
    return y.astype(np.float32), attn.astype(np.float32)


def run_traced(inputs):
    """Like kernel() but with NTFF tracing; returns BassKernelResults."""
    nc, in_maps = _prepare(**inputs)
    return run_bass_kernel_spmd(nc, in_maps, list(range(B)), trace=True)


# revision 36
# speedup vs baseline: 1.1281x; 1.1281x over previous
"""Trainium2 Bass kernel for nn_DirectionalAttention (B=8,S=1024,D=1024,H=16).

Sharding: pure data-parallel over batch — 8 batch elements onto 8 NeuronCores,
zero collectives; each core runs the full attention layer for one batch
element. Host pre-transposes activations/weights (no on-device input
transposes) and folds the out-proj bias as bo' = bv@wo.T + bo into the
residual input (valid because softmax rows sum to 1).

All matmul operands are fp16 (PSUM accumulates fp32; measured end-to-end
error ~5e-4 relmax on attn, ~3e-4 on y). Per core:
  QT = wq @ x^T [D,S], KT likewise, V = v @ wv^T [S,D]  (k-outer, 8 live
    PSUM banks, weights resident, activations streamed twice)
  Attention runs over head PAIRS (2j, 2j+1) whose DK=64 slices sit at PE
  row groups 0:64/64:128, interleaving their K=64 matmuls for array-level
  concurrency, with a depth-1 pair skew so PE stays busy across ACT exp:
    pass1: scores[q,k] -> additive mask (mixed blocks) -> ACT
      Exp(scale=1/8, accum_out=rowsum) -> reciprocal -> normalize (DVE 2x)
      -> DMA out. Fully-masked blocks are skipped entirely (output buffers
      arrive pre-zeroed through the PJRT donation path).
    scoresT is computed directly on PE by swapping matmul operands (no
      transposes), exp'd into fp16 attnT (unnormalized).
    AV is computed transposed, AVT[d,q] = Vh.T @ attnT — exactly the lhsT
      layout the output projection needs; the softmax 1/rowsum is applied
      at AV-copyback via a PE-transposed rinv row broadcast with gpsimd.
      Odd heads reach AVT partitions 64:128 via a small SBUF->SBUF DMA
      bounce (matmul PSUM dst must start at partition 0).
  y = LN(AVT.T @ woT + query + bo') * gamma + beta  (bn_stats/bn_aggr).

The traced program adapts to the causal_mask block structure at build time
(all-ones / all-zero / mixed 128x128 blocks), so any block-structured mask
compiles to a specialized kernel; the tril mask gives the causal program.

Self-contained: hardcodes shapes; reads nothing from /root/problem.
"""

import numpy as np

import concourse.bacc as bacc
import concourse.bass as bass
import concourse.mybir as mybir
from concourse.tile import TileContext
from concourse.masks import make_identity
from concourse.bass_utils import run_bass_kernel_spmd

B, S, D, H = 8, 1024, 1024, 16
DK = D // H
P = 128
NT = S // P
LN_EPS = 1e-5
NEG = -1e9

MM_DT = mybir.dt.float16
F32 = mybir.dt.float32
FP16 = mybir.dt.float16
AFT = mybir.ActivationFunctionType

_CACHE: dict = {}


def _classify_blocks(mask: np.ndarray):
    kinds, mixed = {}, {}
    for qi in range(NT):
        for ki in range(NT):
            blk = mask[qi * P:(qi + 1) * P, ki * P:(ki + 1) * P]
            if (blk != 0).all():
                kinds[qi, ki] = "one"
            elif (blk == 0).all():
                kinds[qi, ki] = "zero"
            else:
                kinds[qi, ki] = "mix"
                mixed[qi, ki] = np.where(blk == 0, np.float32(NEG),
                                         np.float32(0.0))
    return kinds, mixed


def _build(kinds, n_mixed):
    nc = bacc.Bacc("TRN2", target_bir_lowering=False, debug=False,
                   num_devices=B)
    dp = nc.declare_dram_parameter
    xT = dp("xT", [D, S], F32, isOutput=False)
    kTd = dp("kT", [D, S], F32, isOutput=False)
    vT = dp("vT", [D, S], F32, isOutput=False)
    xq = dp("xq", [S, D], F32, isOutput=False)
    wqT = dp("wqT", [D, D], F32, isOutput=False)
    wkT = dp("wkT", [D, D], F32, isOutput=False)
    wvT = dp("wvT", [D, D], F32, isOutput=False)
    woT = dp("woT", [D, D], F32, isOutput=False)
    bq8 = dp("bq8", [P, NT], F32, isOutput=False)
    bk8 = dp("bk8", [P, NT], F32, isOutput=False)
    gam = dp("gam", [1, D], F32, isOutput=False)
    bet = dp("bet", [1, D], F32, isOutput=False)
    mixm = dp("mixm", [max(n_mixed, 1), P, P], F32, isOutput=False)
    mixmT = dp("mixmT", [max(n_mixed, 1), P, P], F32, isOutput=False)
    y_d = dp("y", [S, D], F32, isOutput=True)
    attn_d = dp("attn", [H, S, S], F32, isOutput=True)

    kblocks = {qi: [ki for ki in range(NT) if kinds[qi, ki] != "zero"]
               for qi in range(NT)}
    for qi in range(NT):
        assert kblocks[qi], "fully-masked row tile: softmax undefined"
    mix_idx = {}
    for qi in range(NT):
        for ki in range(NT):
            if kinds[qi, ki] == "mix":
                mix_idx[qi, ki] = len(mix_idx)

    with TileContext(nc) as tc:
      with tc.tile_pool(name="consts", bufs=1) as cp, \
           tc.tile_pool(name="stat", bufs=6) as st:
        ident = cp.tile([P, P], F32)
        make_identity(nc, ident)
        gam_b = cp.tile([P, D], F32)
        bet_b = cp.tile([P, D], F32)
        with tc.tile_pool(name="rowstage", bufs=1) as rsp:
            grow = rsp.tile([1, D], F32, tag="grow")
            brow = rsp.tile([1, D], F32, tag="brow")
            nc.sync.dma_start(grow, gam[:, :])
            nc.sync.dma_start(brow, bet[:, :])
            nc.gpsimd.partition_broadcast(gam_b, grow)
            nc.gpsimd.partition_broadcast(bet_b, brow)
        bq_sb = cp.tile([P, NT], F32)
        bk_sb = cp.tile([P, NT], F32)
        nc.sync.dma_start(bq_sb, bq8[:, :])
        nc.sync.dma_start(bk_sb, bk8[:, :])
        eps_sb = cp.tile([P, 1], F32)
        nc.vector.memset(eps_sb, LN_EPS)
        masks_sb = cp.tile([P, max(n_mixed, 1), P], F32)
        nc.sync.dma_start(masks_sb, mixm[:, :, :].rearrange("n p m -> p n m"))
        masksT_sb = cp.tile([P, max(n_mixed, 1), P], F32)
        nc.sync.dma_start(masksT_sb, mixmT[:, :, :].rearrange("n p m -> p n m"))

        with tc.tile_pool(name="avt", bufs=1) as ap:
          AVT = ap.tile([P, NT, S], FP16)

          with tc.tile_pool(name="qkv", bufs=1) as qp:
            QT = qp.tile([P, NT, S], MM_DT)
            KT = qp.tile([P, NT, S], MM_DT)
            V = qp.tile([P, NT, D], FP16)

            # ============ projections (k-outer, 8 live psum banks) ========
            with tc.tile_pool(name="wres", bufs=1) as wr, \
                 tc.tile_pool(name="praw", bufs=3) as raw, \
                 tc.tile_pool(name="pcast", bufs=3) as cst, \
                 tc.tile_pool(name="ppsum", bufs=8, space="PSUM") as pps:

                def project(w_dram, x_dram, out_sb, bias_sb, w_is_lhsT):
                    w_r = wr.tile([P, NT, D], MM_DT, tag="wres")
                    for k in range(NT):
                        wt = raw.tile([P, D], F32, tag="rawW")
                        nc.sync.dma_start(wt, w_dram[bass.ds(k * P, P), :])
                        nc.vector.tensor_copy(w_r[:, k], wt)
                    for mg in range(2):
                        pss = [pps.tile([P, 512], F32, tag="pj",
                                        name=f"pj{i}")
                               for i in range(8)]
                        for k in range(NT):
                            xt = raw.tile([P, S], F32, tag="rawX")
                            nc.sync.dma_start(
                                xt, x_dram[bass.ds(k * P, P), :])
                            xc = cst.tile([P, S], MM_DT, tag="castX")
                            nc.vector.tensor_copy(xc, xt)
                            for mi in range(4):
                                m = mg * 4 + mi
                                for n in range(2):
                                    if w_is_lhsT:
                                        lhsT = w_r[:, k, m * P:(m + 1) * P]
                                        rhs = xc[:, n * 512:(n + 1) * 512]
                                    else:
                                        lhsT = xc[:, m * P:(m + 1) * P]
                                        rhs = w_r[:, k,
                                                  n * 512:(n + 1) * 512]
                                    nc.tensor.matmul(
                                        pss[mi * 2 + n], lhsT, rhs,
                                        start=(k == 0),
                                        stop=(k == NT - 1))
                        for mi in range(4):
                            m = mg * 4 + mi
                            for n in range(2):
                                dst = out_sb[:, m, n * 512:(n + 1) * 512]
                                ps = pss[mi * 2 + n]
                                if bias_sb is not None:
                                    nc.vector.tensor_scalar_add(
                                        dst, ps, bias_sb[:, m:m + 1])
                                else:
                                    nc.vector.tensor_copy(dst, ps)

                project(wqT, xT, QT, bq_sb, True)
                project(wkT, kTd, KT, bk_sb, True)
                project(wvT, vT, V, None, False)

            # ===================== attention ==============================
            with tc.tile_pool(name="atile", bufs=1) as tp, \
                 tc.tile_pool(name="awork", bufs=3) as wkp, \
                 tc.tile_pool(name="psc", bufs=2, space="PSUM") as psc, \
                 tc.tile_pool(name="pav", bufs=2, space="PSUM") as pavp:
                attnT = tp.tile([P, NT, S], FP16)
                rinv_hq = tp.tile([P, H * NT], F32)
                zero_f = tp.tile([P, S], F32, tag="zf")
                nc.vector.memset(zero_f, 0.0)
                for ki in range(NT):
                    if any(kinds[qi, ki] == "zero" for qi in range(NT)):
                        nc.vector.tensor_copy(attnT[:, ki], zero_f)

                # Head-PAIR loop: heads (2j, 2j+1) sit at PE row
                # groups 0:64 / 64:128, so interleaving their K=64
                # matmuls runs them concurrently in the array (measured
                # ~2x). Depth-1 skew keeps PE dense across ACT exp
                # latency; attnT is (pair x skew) buffered.
                attnT_bufs = [attnT] + [
                    tp.tile([P, NT, S], FP16, name=f"attnT{i}")
                    for i in range(1, 4)]
                for buf in attnT_bufs:
                    for ki in range(NT):
                        if any(kinds[qi, ki] == "zero"
                               for qi in range(NT)):
                            nc.vector.tensor_copy(buf[:, ki], zero_f)

                def emit_pass1_pair(j):
                    for qi in range(NT):
                        kbs = kblocks[qi]
                        wmax = (max(kbs) + 1) * P
                        pss = []
                        for h in (2 * j, 2 * j + 1):
                            ht, hp = divmod(h * DK, P)
                            ps = psc.tile([P, S], F32, tag="psc",
                                          name="ps")
                            pss.append(ps)
                        # interleave the two heads' score matmuls
                        runs = []
                        for g in range(2):
                            grp = [ki for ki in kbs
                                   if g * 4 <= ki < g * 4 + 4]
                            while grp:
                                a = grp[0]
                                b2 = a
                                while b2 + 1 in grp:
                                    b2 += 1
                                grp = [x for x in grp if x > b2]
                                runs.append((a, b2))
                        for (a, b2) in runs:
                            for i, h in enumerate((2 * j, 2 * j + 1)):
                                ht, hp = divmod(h * DK, P)
                                nc.tensor.matmul(
                                    pss[i][:, a * P:(b2 + 1) * P],
                                    QT[hp:hp + DK, ht,
                                       qi * P:(qi + 1) * P],
                                    KT[hp:hp + DK, ht,
                                       a * P:(b2 + 1) * P],
                                    start=True, stop=True)
                        for i, h in enumerate((2 * j, 2 * j + 1)):
                            ps = pss[i]
                            for ki in range(wmax // P):
                                if ki not in kbs:
                                    nc.vector.memset(
                                        ps[:, ki * P:(ki + 1) * P], NEG)
                                elif kinds[qi, ki] == "mix":
                                    nc.vector.tensor_add(
                                        ps[:, ki * P:(ki + 1) * P],
                                        ps[:, ki * P:(ki + 1) * P],
                                        masks_sb[:, mix_idx[qi, ki]])
                            ex = wkp.tile([P, S], F32, tag="exp",
                                          name="ex")
                            rs = st.tile([P, 1], F32, tag="rs", name="rs")
                            at = wkp.tile([P, S], F32, tag="attn",
                                          name="at")
                            nc.scalar.activation(
                                ex[:, :wmax], ps[:, :wmax], AFT.Exp,
                                scale=0.125, accum_out=rs)
                            hq = h * NT + qi
                            nc.vector.reciprocal(rinv_hq[:, hq:hq + 1], rs)
                            nc.vector.tensor_scalar_mul(
                                at[:, :wmax], ex[:, :wmax],
                                rinv_hq[:, hq:hq + 1])
                            nc.sync.dma_start(
                                attn_d[h, bass.ds(qi * P, P), 0:wmax],
                                at[:, :wmax])

                def emit_rbc_pair(j):
                    pstr = pavp.tile([P, 512], F32, tag="psT",
                                     name="pstr")
                    nc.tensor.transpose(pstr[:, 0:P], rinv_hq, ident)
                    rt = tp.tile([P, P], F32, tag="rinvT", name="rt",
                                 bufs=2)
                    nc.vector.tensor_copy(rt, pstr[:, 0:P])
                    rbcs = {}
                    for h in (2 * j, 2 * j + 1):
                        for c in range(2):
                            hq0 = h * NT + 4 * c
                            rstage = wkp.tile([1, 512], F32, tag="rstg",
                                              name="rstage")
                            nc.gpsimd.dma_start(rstage, rt[hq0:hq0 + 4, :])
                            rbc = wkp.tile([DK, 512], F32, tag="rbc",
                                           name="rbc", bufs=8)
                            nc.gpsimd.partition_broadcast(rbc, rstage)
                            rbcs[h, c] = rbc
                    return rbcs

                def emit_scoresT_pair(j, bufs2):
                    for c in range(2):
                        c0 = c * 512
                        for ki in range(NT):
                            qs = [qi for qi in range(4 * c, 4 * c + 4)
                                  if kinds[qi, ki] != "zero"]
                            if not qs:
                                continue
                            if qs != list(range(qs[0], qs[-1] + 1)):
                                raise NotImplementedError(
                                    "non-contiguous valid q-blocks")
                            qlo = qs[0] * P - c0
                            qhi = (qs[-1] + 1) * P - c0
                            psTs = []
                            for i, h in enumerate((2 * j, 2 * j + 1)):
                                ht, hp = divmod(h * DK, P)
                                psT = pavp.tile([P, 512], F32, tag="psT",
                                                name="psT")
                                nc.tensor.matmul(
                                    psT[:, qlo:qhi],
                                    KT[hp:hp + DK, ht,
                                       ki * P:(ki + 1) * P],
                                    QT[hp:hp + DK, ht,
                                       c0 + qlo:c0 + qhi],
                                    start=True, stop=True)
                                psTs.append(psT)
                            for i, h in enumerate((2 * j, 2 * j + 1)):
                                psT = psTs[i]
                                for qi in qs:
                                    a = qi * P - c0
                                    if kinds[qi, ki] == "mix":
                                        nc.vector.tensor_add(
                                            psT[:, a:a + P],
                                            psT[:, a:a + P],
                                            masksT_sb[:, mix_idx[qi, ki]])
                                nc.scalar.activation(
                                    bufs2[i][:, ki, c0 + qlo:c0 + qhi],
                                    psT[:, qlo:qhi], AFT.Exp, scale=0.125)

                def emit_av_pair(j, bufs2, rbcs):
                    for c in range(2):
                        kis = sorted({k for qi in range(4 * c, 4 * c + 4)
                                      for k in kblocks[qi]})
                        pas = []
                        for i, h in enumerate((2 * j, 2 * j + 1)):
                            pa = pavp.tile([DK, 512], F32, tag="pav",
                                           name="pa", bufs=2)
                            pas.append(pa)
                        for idx, ki in enumerate(kis):
                            for i, h in enumerate((2 * j, 2 * j + 1)):
                                nc.tensor.matmul(
                                    pas[i],
                                    V[:, ki, h * DK:(h + 1) * DK],
                                    bufs2[i][:, ki,
                                             c * 512:(c + 1) * 512],
                                    start=(idx == 0),
                                    stop=(idx == len(kis) - 1))
                        for i, h in enumerate((2 * j, 2 * j + 1)):
                            ht, hp = divmod(h * DK, P)
                            if hp == 0:
                                nc.vector.tensor_mul(
                                    AVT[0:DK, ht,
                                        c * 512:(c + 1) * 512],
                                    pas[i], rbcs[h, c])
                            else:
                                bounce = wkp.tile([DK, 512], FP16,
                                                  tag="bnc", name="bounce")
                                nc.vector.tensor_mul(
                                    bounce, pas[i], rbcs[h, c])
                                nc.sync.dma_start(
                                    AVT[hp:hp + DK, ht,
                                        c * 512:(c + 1) * 512], bounce)

                prev = None
                for j in range(H // 2):
                    bufs2 = [attnT_bufs[2 * (j % 2)],
                             attnT_bufs[2 * (j % 2) + 1]]
                    emit_pass1_pair(j)
                    rbcs = emit_rbc_pair(j)
                    emit_scoresT_pair(j, bufs2)
                    if prev is not None:
                        emit_av_pair(*prev)
                    prev = (j, bufs2, rbcs)
                assert prev is not None
                emit_av_pair(*prev)

          # ============== output projection + residual + LN ===============
          with tc.tile_pool(name="ores", bufs=1) as owr, \
               tc.tile_pool(name="oraw", bufs=3) as oraw, \
               tc.tile_pool(name="owork", bufs=2) as ow, \
               tc.tile_pool(name="opsum", bufs=4, space="PSUM") as ops:
            wo_r = owr.tile([P, NT, D], FP16)
            for k in range(NT):
                wt = oraw.tile([P, D], F32, tag="rawW")
                nc.sync.dma_start(wt, woT[bass.ds(k * P, P), :])
                nc.vector.tensor_copy(wo_r[:, k], wt)
            for m in range(NT):
                xr = oraw.tile([P, D], F32, tag="rawX")
                nc.sync.dma_start(xr, xq[bass.ds(m * P, P), :])
                y0 = ow.tile([P, D], F32, tag="y0")
                for n in range(2):
                    ps = ops.tile([P, 512], F32, tag="oj")
                    for k in range(NT):
                        nc.tensor.matmul(
                            ps, AVT[:, k, m * P:(m + 1) * P],
                            wo_r[:, k, n * 512:(n + 1) * 512],
                            start=(k == 0), stop=(k == NT - 1))
                    nc.vector.tensor_add(
                        y0[:, n * 512:(n + 1) * 512], ps,
                        xr[:, n * 512:(n + 1) * 512])
                stats = st.tile([P, 2, 6], F32, tag="bns")
                mv = st.tile([P, 2], F32, tag="mv")
                for sg in range(2):
                    nc.vector.bn_stats(stats[:, sg],
                                       y0[:, sg * 512:(sg + 1) * 512])
                nc.vector.bn_aggr(mv, stats)
                sd = st.tile([P, 1], F32, tag="sd")
                nc.scalar.activation(sd, mv[:, 1:2], AFT.Sqrt, bias=eps_sb)
                rsd = st.tile([P, 1], F32, tag="rsd")
                nc.vector.reciprocal(rsd, sd)
                y1 = ow.tile([P, D], F32, tag="y1")
                nc.vector.tensor_scalar(
                    y1, y0, scalar1=mv[:, 0:1], scalar2=rsd,
                    op0=mybir.AluOpType.subtract, op1=mybir.AluOpType.mult)
                nc.vector.tensor_mul(y1, y1, gam_b)
                nc.gpsimd.tensor_add(y1, y1, bet_b)
                nc.sync.dma_start(y_d[bass.ds(m * P, P), :], y1)

    nc.compile()
    return nc


def _prepare(query, key, value, causal_mask, wq, bq, wk, bk, wv, bv,
             wo, bo, gamma, beta):
    query = np.asarray(query, np.float32)
    key = np.asarray(key, np.float32)
    value = np.asarray(value, np.float32)
    mask = np.asarray(causal_mask)
    wq, bq = np.asarray(wq, np.float32), np.asarray(bq, np.float32)
    wk, bk = np.asarray(wk, np.float32), np.asarray(bk, np.float32)
    wv, bv = np.asarray(wv, np.float32), np.asarray(bv, np.float32)
    wo, bo = np.asarray(wo, np.float32), np.asarray(bo, np.float32)
    gamma, beta = np.asarray(gamma, np.float32), np.asarray(beta, np.float32)

    kinds, mixed = _classify_blocks(mask)
    key_sig = tuple(sorted(kinds.items()))
    if key_sig not in _CACHE:
        _CACHE[key_sig] = _build(kinds, max(len(mixed), 1))
    nc = _CACHE[key_sig]

    mix_list = [mixed[qi, ki] for qi in range(NT) for ki in range(NT)
                if kinds[qi, ki] == "mix"]
    mixm = (np.stack(mix_list) if mix_list
            else np.zeros((1, P, P), np.float32))
    mixmT = np.ascontiguousarray(mixm.transpose(0, 2, 1))
    bo_p = (bv @ wo.T + bo).astype(np.float32)
    shared = {
        "wqT": np.ascontiguousarray(wq.T), "wkT": np.ascontiguousarray(wk.T),
        "wvT": np.ascontiguousarray(wv.T), "woT": np.ascontiguousarray(wo.T),
        "bq8": np.ascontiguousarray(bq.reshape(NT, P).T),
        "bk8": np.ascontiguousarray(bk.reshape(NT, P).T),
        "gam": np.ascontiguousarray(gamma.reshape(1, D)),
        "bet": np.ascontiguousarray(beta.reshape(1, D)),
        "mixm": np.ascontiguousarray(mixm),
        "mixmT": mixmT,
    }
    in_maps = [{
        "xT": np.ascontiguousarray(query[b].T),
        "kT": np.ascontiguousarray(key[b].T),
        "vT": np.ascontiguousarray(value[b].T),
        "xq": np.ascontiguousarray(query[b] + bo_p),
        **shared,
    } for b in range(B)]
    return nc, in_maps


def kernel(**inputs):
    nc, in_maps = _prepare(**inputs)
    res = run_bass_kernel_spmd(nc, in_maps, list(range(B)))
    y = np.stack([res.results[b]["y"] for b in range(B)])
    attn = np.stack([res.results[b]["attn"] for b in range(B)])
    return y.astype(np.float32), attn.astype(np.float32)


def run_traced(inputs):
    """Like kernel() but with NTFF tracing; returns BassKernelResults."""
    nc, in_maps = _prepare(**inputs)
    return run_bass_kernel_spmd(nc, in_maps, list(range(B)), trace=True)


# revision 37
# speedup vs baseline: 1.2033x; 1.0667x over previous
"""Trainium2 Bass kernel for nn_DirectionalAttention (B=8,S=1024,D=1024,H=16).

Sharding: pure data-parallel over batch — 8 batch elements onto 8 NeuronCores,
zero collectives; each core runs the full attention layer for one batch
element. Host pre-transposes activations/weights (no on-device input
transposes) and folds the out-proj bias as bo' = bv@wo.T + bo into the
residual input (valid because softmax rows sum to 1).

All matmul operands are fp16 (PSUM accumulates fp32; measured end-to-end
error ~5e-4 relmax on attn, ~3e-4 on y). Per core:
  QT = wq @ x^T [D,S], KT likewise, V = v @ wv^T [S,D]  (k-outer, 8 live
    PSUM banks, weights resident, activations streamed twice)
  Attention runs over head PAIRS (2j, 2j+1) whose DK=64 slices sit at PE
  row groups 0:64/64:128, interleaving their K=64 matmuls for array-level
  concurrency, with a depth-1 pair skew so PE stays busy across ACT exp:
    pass1: scores[q,k] -> additive mask (mixed blocks) -> ACT
      Exp(scale=1/8, accum_out=rowsum) -> reciprocal -> normalize (DVE 2x)
      -> DMA out. Fully-masked blocks are skipped entirely (output buffers
      arrive pre-zeroed through the PJRT donation path).
    scoresT is computed directly on PE by swapping matmul operands (no
      transposes), exp'd into fp16 attnT (unnormalized).
    AV is computed transposed, AVT[d,q] = Vh.T @ attnT — exactly the lhsT
      layout the output projection needs; the softmax 1/rowsum is applied
      at AV-copyback via a PE-transposed rinv row broadcast with gpsimd.
      Odd heads reach AVT partitions 64:128 via a small SBUF->SBUF DMA
      bounce (matmul PSUM dst must start at partition 0).
  y = LN(AVT.T @ woT + query + bo') * gamma + beta  (bn_stats/bn_aggr).

The traced program adapts to the causal_mask block structure at build time
(all-ones / all-zero / mixed 128x128 blocks), so any block-structured mask
compiles to a specialized kernel; the tril mask gives the causal program.

Self-contained: hardcodes shapes; reads nothing from /root/problem.
"""

import numpy as np

import concourse.bacc as bacc
import concourse.bass as bass
import concourse.mybir as mybir
from concourse.tile import TileContext
from concourse.masks import make_identity
from concourse.bass_utils import run_bass_kernel_spmd

B, S, D, H = 8, 1024, 1024, 16
DK = D // H
P = 128
NT = S // P
LN_EPS = 1e-5
NEG = -1e9

MM_DT = mybir.dt.float16
F32 = mybir.dt.float32
FP16 = mybir.dt.float16
AFT = mybir.ActivationFunctionType

_CACHE: dict = {}


def _classify_blocks(mask: np.ndarray):
    kinds, mixed = {}, {}
    for qi in range(NT):
        for ki in range(NT):
            blk = mask[qi * P:(qi + 1) * P, ki * P:(ki + 1) * P]
            if (blk != 0).all():
                kinds[qi, ki] = "one"
            elif (blk == 0).all():
                kinds[qi, ki] = "zero"
            else:
                kinds[qi, ki] = "mix"
                mixed[qi, ki] = np.where(blk == 0, np.float32(NEG),
                                         np.float32(0.0))
    return kinds, mixed


def _build(kinds, n_mixed):
    nc = bacc.Bacc("TRN2", target_bir_lowering=False, debug=False,
                   num_devices=B)
    dp = nc.declare_dram_parameter
    xT = dp("xT", [D, S], F32, isOutput=False)
    kTd = dp("kT", [D, S], F32, isOutput=False)
    vT = dp("vT", [D, S], F32, isOutput=False)
    xq = dp("xq", [S, D], F32, isOutput=False)
    wqT = dp("wqT", [D, D], F32, isOutput=False)
    wkT = dp("wkT", [D, D], F32, isOutput=False)
    wvT = dp("wvT", [D, D], F32, isOutput=False)
    woT = dp("woT", [D, D], F32, isOutput=False)
    bq8 = dp("bq8", [P, NT], F32, isOutput=False)
    bk8 = dp("bk8", [P, NT], F32, isOutput=False)
    gam = dp("gam", [1, D], F32, isOutput=False)
    bet = dp("bet", [1, D], F32, isOutput=False)
    mixm = dp("mixm", [max(n_mixed, 1), P, P], F32, isOutput=False)
    mixmT = dp("mixmT", [max(n_mixed, 1), P, P], F32, isOutput=False)
    y_d = dp("y", [S, D], F32, isOutput=True)
    attn_d = dp("attn", [H, S, S], F32, isOutput=True)

    kblocks = {qi: [ki for ki in range(NT) if kinds[qi, ki] != "zero"]
               for qi in range(NT)}
    for qi in range(NT):
        assert kblocks[qi], "fully-masked row tile: softmax undefined"
    mix_idx = {}
    for qi in range(NT):
        for ki in range(NT):
            if kinds[qi, ki] == "mix":
                mix_idx[qi, ki] = len(mix_idx)

    with TileContext(nc) as tc:
      with tc.tile_pool(name="consts", bufs=1) as cp, \
           tc.tile_pool(name="stat", bufs=6) as st:
        ident = cp.tile([P, P], F32)
        make_identity(nc, ident)
        gam_b = cp.tile([P, D], F32)
        bet_b = cp.tile([P, D], F32)
        with tc.tile_pool(name="rowstage", bufs=1) as rsp:
            grow = rsp.tile([1, D], F32, tag="grow")
            brow = rsp.tile([1, D], F32, tag="brow")
            nc.sync.dma_start(grow, gam[:, :])
            nc.sync.dma_start(brow, bet[:, :])
            nc.gpsimd.partition_broadcast(gam_b, grow)
            nc.gpsimd.partition_broadcast(bet_b, brow)
        bq_sb = cp.tile([P, NT], F32)
        bk_sb = cp.tile([P, NT], F32)
        nc.sync.dma_start(bq_sb, bq8[:, :])
        nc.sync.dma_start(bk_sb, bk8[:, :])
        eps_sb = cp.tile([P, 1], F32)
        nc.vector.memset(eps_sb, LN_EPS)
        masks_sb = cp.tile([P, max(n_mixed, 1), P], F32)
        nc.sync.dma_start(masks_sb, mixm[:, :, :].rearrange("n p m -> p n m"))
        masksT_sb = cp.tile([P, max(n_mixed, 1), P], F32)
        nc.sync.dma_start(masksT_sb, mixmT[:, :, :].rearrange("n p m -> p n m"))

        with tc.tile_pool(name="avt", bufs=1) as ap:
          AVT = ap.tile([P, NT, S], FP16)

          with tc.tile_pool(name="qkv", bufs=1) as qp:
            QT = qp.tile([P, NT, S], MM_DT)
            KT = qp.tile([P, NT, S], MM_DT)
            V = qp.tile([P, NT, D], FP16)

            # ============ projections (k-outer, 8 live psum banks) ========
            with tc.tile_pool(name="wres", bufs=1) as wr, \
                 tc.tile_pool(name="praw", bufs=5) as raw, \
                 tc.tile_pool(name="pcast", bufs=5) as cst, \
                 tc.tile_pool(name="ppsum", bufs=8, space="PSUM") as pps:

                def project(w_dram, x_dram, out_sb, bias_sb, w_is_lhsT):
                    w_r = wr.tile([P, NT, D], MM_DT, tag="wres")
                    for k in range(NT):
                        wt = raw.tile([P, D], F32, tag="rawW")
                        nc.sync.dma_start(wt, w_dram[bass.ds(k * P, P), :])
                        nc.vector.tensor_copy(w_r[:, k], wt)
                    for mg in range(2):
                        pss = [pps.tile([P, 512], F32, tag="pj",
                                        name=f"pj{i}")
                               for i in range(8)]
                        for k in range(NT):
                            xt = raw.tile([P, S], F32, tag="rawX")
                            nc.sync.dma_start(
                                xt, x_dram[bass.ds(k * P, P), :])
                            xc = cst.tile([P, S], MM_DT, tag="castX")
                            nc.vector.tensor_copy(xc, xt)
                            for mi in range(4):
                                m = mg * 4 + mi
                                for n in range(2):
                                    if w_is_lhsT:
                                        lhsT = w_r[:, k, m * P:(m + 1) * P]
                                        rhs = xc[:, n * 512:(n + 1) * 512]
                                    else:
                                        lhsT = xc[:, m * P:(m + 1) * P]
                                        rhs = w_r[:, k,
                                                  n * 512:(n + 1) * 512]
                                    nc.tensor.matmul(
                                        pss[mi * 2 + n], lhsT, rhs,
                                        start=(k == 0),
                                        stop=(k == NT - 1))
                        for mi in range(4):
                            m = mg * 4 + mi
                            for n in range(2):
                                dst = out_sb[:, m, n * 512:(n + 1) * 512]
                                ps = pss[mi * 2 + n]
                                if bias_sb is not None:
                                    nc.vector.tensor_scalar_add(
                                        dst, ps, bias_sb[:, m:m + 1])
                                else:
                                    nc.vector.tensor_copy(dst, ps)

                project(wqT, xT, QT, bq_sb, True)
                project(wkT, kTd, KT, bk_sb, True)
                project(wvT, vT, V, None, False)

            # ===================== attention ==============================
            with tc.tile_pool(name="atile", bufs=1) as tp, \
                 tc.tile_pool(name="awork", bufs=3) as wkp, \
                 tc.tile_pool(name="psc", bufs=2, space="PSUM") as psc, \
                 tc.tile_pool(name="pav", bufs=2, space="PSUM") as pavp:
                attnT = tp.tile([P, NT, S], FP16)
                rinv_hq = tp.tile([P, H * NT], F32)
                zero_f = tp.tile([P, S], F32, tag="zf")
                nc.vector.memset(zero_f, 0.0)
                for ki in range(NT):
                    if any(kinds[qi, ki] == "zero" for qi in range(NT)):
                        nc.vector.tensor_copy(attnT[:, ki], zero_f)

                # Head-PAIR loop: heads (2j, 2j+1) sit at PE row
                # groups 0:64 / 64:128, so interleaving their K=64
                # matmuls runs them concurrently in the array (measured
                # ~2x). Depth-1 skew keeps PE dense across ACT exp
                # latency; attnT is (pair x skew) buffered.
                attnT_bufs = [attnT] + [
                    tp.tile([P, NT, S], FP16, name=f"attnT{i}")
                    for i in range(1, 4)]
                for buf in attnT_bufs:
                    for ki in range(NT):
                        if any(kinds[qi, ki] == "zero"
                               for qi in range(NT)):
                            nc.vector.tensor_copy(buf[:, ki], zero_f)

                def emit_pass1_pair(j):
                    for qi in range(NT):
                        kbs = kblocks[qi]
                        wmax = (max(kbs) + 1) * P
                        pss = []
                        for h in (2 * j, 2 * j + 1):
                            ht, hp = divmod(h * DK, P)
                            ps = psc.tile([P, S], F32, tag="psc",
                                          name="ps")
                            pss.append(ps)
                        # interleave the two heads' score matmuls
                        runs = []
                        for g in range(2):
                            grp = [ki for ki in kbs
                                   if g * 4 <= ki < g * 4 + 4]
                            while grp:
                                a = grp[0]
                                b2 = a
                                while b2 + 1 in grp:
                                    b2 += 1
                                grp = [x for x in grp if x > b2]
                                runs.append((a, b2))
                        for (a, b2) in runs:
                            for i, h in enumerate((2 * j, 2 * j + 1)):
                                ht, hp = divmod(h * DK, P)
                                nc.tensor.matmul(
                                    pss[i][:, a * P:(b2 + 1) * P],
                                    QT[hp:hp + DK, ht,
                                       qi * P:(qi + 1) * P],
                                    KT[hp:hp + DK, ht,
                                       a * P:(b2 + 1) * P],
                                    start=True, stop=True)
                        for i, h in enumerate((2 * j, 2 * j + 1)):
                            ps = pss[i]
                            for ki in range(wmax // P):
                                if ki not in kbs:
                                    nc.vector.memset(
                                        ps[:, ki * P:(ki + 1) * P], NEG)
                                elif kinds[qi, ki] == "mix":
                                    nc.vector.tensor_add(
                                        ps[:, ki * P:(ki + 1) * P],
                                        ps[:, ki * P:(ki + 1) * P],
                                        masks_sb[:, mix_idx[qi, ki]])
                            ex = wkp.tile([P, S], F32, tag="exp",
                                          name="ex")
                            rs = st.tile([P, 1], F32, tag="rs", name="rs")
                            at = wkp.tile([P, S], F32, tag="attn",
                                          name="at")
                            nc.scalar.activation(
                                ex[:, :wmax], ps[:, :wmax], AFT.Exp,
                                scale=0.125, accum_out=rs)
                            hq = h * NT + qi
                            nc.vector.reciprocal(rinv_hq[:, hq:hq + 1], rs)
                            nc.vector.tensor_scalar_mul(
                                at[:, :wmax], ex[:, :wmax],
                                rinv_hq[:, hq:hq + 1])
                            nc.sync.dma_start(
                                attn_d[h, bass.ds(qi * P, P), 0:wmax],
                                at[:, :wmax])

                def emit_rbc_pair(j):
                    pstr = pavp.tile([P, 512], F32, tag="psT",
                                     name="pstr")
                    nc.tensor.transpose(pstr[:, 0:P], rinv_hq, ident)
                    rt = tp.tile([P, P], F32, tag="rinvT", name="rt",
                                 bufs=2)
                    nc.vector.tensor_copy(rt, pstr[:, 0:P])
                    rbcs = {}
                    for h in (2 * j, 2 * j + 1):
                        for c in range(2):
                            hq0 = h * NT + 4 * c
                            rstage = wkp.tile([1, 512], F32, tag="rstg",
                                              name="rstage")
                            nc.gpsimd.dma_start(rstage, rt[hq0:hq0 + 4, :])
                            rbc = wkp.tile([DK, 512], F32, tag="rbc",
                                           name="rbc", bufs=8)
                            nc.gpsimd.partition_broadcast(rbc, rstage)
                            rbcs[h, c] = rbc
                    return rbcs

                def emit_scoresT_pair(j, bufs2):
                    for c in range(2):
                        c0 = c * 512
                        for ki in range(NT):
                            qs = [qi for qi in range(4 * c, 4 * c + 4)
                                  if kinds[qi, ki] != "zero"]
                            if not qs:
                                continue
                            if qs != list(range(qs[0], qs[-1] + 1)):
                                raise NotImplementedError(
                                    "non-contiguous valid q-blocks")
                            qlo = qs[0] * P - c0
                            qhi = (qs[-1] + 1) * P - c0
                            psTs = []
                            for i, h in enumerate((2 * j, 2 * j + 1)):
                                ht, hp = divmod(h * DK, P)
                                psT = pavp.tile([P, 512], F32, tag="psT",
                                                name="psT")
                                nc.tensor.matmul(
                                    psT[:, qlo:qhi],
                                    KT[hp:hp + DK, ht,
                                       ki * P:(ki + 1) * P],
                                    QT[hp:hp + DK, ht,
                                       c0 + qlo:c0 + qhi],
                                    start=True, stop=True)
                                psTs.append(psT)
                            for i, h in enumerate((2 * j, 2 * j + 1)):
                                psT = psTs[i]
                                for qi in qs:
                                    a = qi * P - c0
                                    if kinds[qi, ki] == "mix":
                                        nc.vector.tensor_add(
                                            psT[:, a:a + P],
                                            psT[:, a:a + P],
                                            masksT_sb[:, mix_idx[qi, ki]])
                                nc.scalar.activation(
                                    bufs2[i][:, ki, c0 + qlo:c0 + qhi],
                                    psT[:, qlo:qhi], AFT.Exp, scale=0.125)

                def emit_av_pair(j, bufs2, rbcs):
                    for c in range(2):
                        kis = sorted({k for qi in range(4 * c, 4 * c + 4)
                                      for k in kblocks[qi]})
                        pas = []
                        for i, h in enumerate((2 * j, 2 * j + 1)):
                            pa = pavp.tile([DK, 512], F32, tag="pav",
                                           name="pa", bufs=2)
                            pas.append(pa)
                        for idx, ki in enumerate(kis):
                            for i, h in enumerate((2 * j, 2 * j + 1)):
                                nc.tensor.matmul(
                                    pas[i],
                                    V[:, ki, h * DK:(h + 1) * DK],
                                    bufs2[i][:, ki,
                                             c * 512:(c + 1) * 512],
                                    start=(idx == 0),
                                    stop=(idx == len(kis) - 1))
                        for i, h in enumerate((2 * j, 2 * j + 1)):
                            ht, hp = divmod(h * DK, P)
                            if hp == 0:
                                nc.vector.tensor_mul(
                                    AVT[0:DK, ht,
                                        c * 512:(c + 1) * 512],
                                    pas[i], rbcs[h, c])
                            else:
                                bounce = wkp.tile([DK, 512], FP16,
                                                  tag="bnc", name="bounce")
                                nc.vector.tensor_mul(
                                    bounce, pas[i], rbcs[h, c])
                                nc.sync.dma_start(
                                    AVT[hp:hp + DK, ht,
                                        c * 512:(c + 1) * 512], bounce)

                prev = None
                for j in range(H // 2):
                    bufs2 = [attnT_bufs[2 * (j % 2)],
                             attnT_bufs[2 * (j % 2) + 1]]
                    emit_pass1_pair(j)
                    rbcs = emit_rbc_pair(j)
                    emit_scoresT_pair(j, bufs2)
                    if prev is not None:
                        emit_av_pair(*prev)
                    prev = (j, bufs2, rbcs)
                assert prev is not None
                emit_av_pair(*prev)

          # ============== output projection + residual + LN ===============
          with tc.tile_pool(name="ores", bufs=1) as owr, \
               tc.tile_pool(name="oraw", bufs=3) as oraw, \
               tc.tile_pool(name="owork", bufs=2) as ow, \
               tc.tile_pool(name="opsum", bufs=4, space="PSUM") as ops:
            wo_r = owr.tile([P, NT, D], FP16)
            for k in range(NT):
                wt = oraw.tile([P, D], F32, tag="rawW")
                nc.sync.dma_start(wt, woT[bass.ds(k * P, P), :])
                nc.vector.tensor_copy(wo_r[:, k], wt)
            for m in range(NT):
                xr = oraw.tile([P, D], F32, tag="rawX")
                nc.sync.dma_start(xr, xq[bass.ds(m * P, P), :])
                y0 = ow.tile([P, D], F32, tag="y0")
                for n in range(2):
                    ps = ops.tile([P, 512], F32, tag="oj")
                    for k in range(NT):
                        nc.tensor.matmul(
                            ps, AVT[:, k, m * P:(m + 1) * P],
                            wo_r[:, k, n * 512:(n + 1) * 512],
                            start=(k == 0), stop=(k == NT - 1))
                    nc.vector.tensor_add(
                        y0[:, n * 512:(n + 1) * 512], ps,
                        xr[:, n * 512:(n + 1) * 512])
                stats = st.tile([P, 2, 6], F32, tag="bns")
                mv = st.tile([P, 2], F32, tag="mv")
                for sg in range(2):
                    nc.vector.bn_stats(stats[:, sg],
                                       y0[:, sg * 512:(sg + 1) * 512])
                nc.vector.bn_aggr(mv, stats)
                sd = st.tile([P, 1], F32, tag="sd")
                nc.scalar.activation(sd, mv[:, 1:2], AFT.Sqrt, bias=eps_sb)
                rsd = st.tile([P, 1], F32, tag="rsd")
                nc.vector.reciprocal(rsd, sd)
                y1 = ow.tile([P, D], F32, tag="y1")
                nc.vector.tensor_scalar(
                    y1, y0, scalar1=mv[:, 0:1], scalar2=rsd,
                    op0=mybir.AluOpType.subtract, op1=mybir.AluOpType.mult)
                nc.vector.tensor_mul(y1, y1, gam_b)
                nc.gpsimd.tensor_add(y1, y1, bet_b)
                nc.sync.dma_start(y_d[bass.ds(m * P, P), :], y1)

    nc.compile()
    return nc


def _prepare(query, key, value, causal_mask, wq, bq, wk, bk, wv, bv,
             wo, bo, gamma, beta):
    query = np.asarray(query, np.float32)
    key = np.asarray(key, np.float32)
    value = np.asarray(value, np.float32)
    mask = np.asarray(causal_mask)
    wq, bq = np.asarray(wq, np.float32), np.asarray(bq, np.float32)
    wk, bk = np.asarray(wk, np.float32), np.asarray(bk, np.float32)
    wv, bv = np.asarray(wv, np.float32), np.asarray(bv, np.float32)
    wo, bo = np.asarray(wo, np.float32), np.asarray(bo, np.float32)
    gamma, beta = np.asarray(gamma, np.float32), np.asarray(beta, np.float32)

    kinds, mixed = _classify_blocks(mask)
    key_sig = tuple(sorted(kinds.items()))
    if key_sig not in _CACHE:
        _CACHE[key_sig] = _build(kinds, max(len(mixed), 1))
    nc = _CACHE[key_sig]

    mix_list = [mixed[qi, ki] for qi in range(NT) for ki in range(NT)
                if kinds[qi, ki] == "mix"]
    mixm = (np.stack(mix_list) if mix_list
            else np.zeros((1, P, P), np.float32))
    mixmT = np.ascontiguousarray(mixm.transpose(0, 2, 1))
    bo_p = (bv @ wo.T + bo).astype(np.float32)
    shared = {
        "wqT": np.ascontiguousarray(wq.T), "wkT": np.ascontiguousarray(wk.T),
        "wvT": np.ascontiguousarray(wv.T), "woT": np.ascontiguousarray(wo.T),
        "bq8": np.ascontiguousarray(bq.reshape(NT, P).T),
        "bk8": np.ascontiguousarray(bk.reshape(NT, P).T),
        "gam": np.ascontiguousarray(gamma.reshape(1, D)),
        "bet": np.ascontiguousarray(beta.reshape(1, D)),
        "mixm": np.ascontiguousarray(mixm),
        "mixmT": mixmT,
    }
    in_maps = [{
        "xT": np.ascontiguousarray(query[b].T),
        "kT": np.ascontiguousarray(key[b].T),
        "vT": np.ascontiguousarray(value[b].T),
        "xq": np.ascontiguousarray(query[b] + bo_p),
        **shared,
    } for b in range(B)]
    return nc, in_maps


def kernel(**inputs):
    nc, in_maps = _prepare(**inputs)
    res = run_bass_kernel_spmd(nc, in_maps, list(range(B)))
    y = np.stack([res.results[b]["y"] for b in range(B)])
    attn = np.stack([res.results[b]["attn"] for b in range(B)])
    return y.astype(np.float32), attn.astype(np.float32)


def run_traced(inputs):
    """Like kernel() but with NTFF tracing; returns BassKernelResults."""
    nc, in_maps = _prepare(**inputs)
    return run_bass_kernel_spmd(nc, in_maps, list(range(B)), trace=True)
